# revision 2
# baseline (speedup 1.0000x reference)
"""Trainium2 Bass kernel for nn_NeuralODEModel (fixed-step Euler neural ODE).

Math (per batch b, all rows n independent):
  y0 = concat([z0, disappear_time], -1)            # [N, D1]
  repeat 9x: 120 Euler steps y += DT * (tanh(y@W1 + b1) @ W2 + b2)
  out[i] = y_after_{120*i}_steps * (i/10 < disappear_time)   # i = 0..9

Sharding: data-parallel across B=8 -> one batch per NeuronCore (SPMD).

Per-core kernel design:
  - State is kept TRANSPOSED in SBUF/PSUM: ST = y^T [D1=128 part, n free],
    so both matmuls contract over the partition dim with weights stationary:
      mm1: psum1[:,j,:] = W1[:,128j:128j+128].T @ ST     (j = 0,1 -> H=256)
      tanh: h = tanh(psum1 (+b1))          (one ACT op over [128, 2, n])
      mm2: psumY += (DT*W2)[128j:,:].T @ h[:,j,:]        (accumulate onto y^T)
      copy: ST' = psumY                    (DVE PSUM->SBUF, rhs for next step)
    psumY is a persistent PSUM accumulator initialized with y0^T by a PE
    transpose, so y^T lives in PSUM and every step just accumulates into it.
  - C row-chains (columns of ST) are stepped in an interleaved order so the
    serial mm1->tanh->mm2->copy dependency of one chain hides under the
    engine work of the others.
  - Snapshots (every 120 steps + t=0): PE-transpose ST back to natural
    [n, D1], multiply by the precomputed per-row mask (DVE tensor_scalar
    with a per-partition operand), DMA to the output.
"""

import os

import numpy as np

import concourse.bacc as bacc
import concourse.mybir as mybir
from concourse import tile
from concourse.bass_utils import run_bass_kernel_spmd

F32 = mybir.dt.float32
AF = mybir.ActivationFunctionType

B, N, D1, H, TS = 8, 128, 128, 256, 10
DT = 1.0 / 1200.0
STEPS_PER_INT = 120

NUM_CHAINS = int(os.environ.get("NODE_CHAINS", "2"))
MM2_DT = os.environ.get("NODE_MM2_DT", "f32")  # f32 | f16 | bf16
_MM2_DTYPE = {
    "f32": mybir.dt.float32,
    "f16": mybir.dt.float16,
    "bf16": mybir.dt.bfloat16,
}


def build_nc(
    zero_b1: bool,
    zero_b2: bool,
    n_outer: int = TS - 1,
    n_steps: int = STEPS_PER_INT,
    chains: int = NUM_CHAINS,
    mm2_dt: str = MM2_DT,
):
    """Build the per-core SPMD Bass program. Returns a compiled Bacc."""
    nc = bacc.Bacc()
    CW = N // chains  # rows per chain
    h_dtype = _MM2_DTYPE[mm2_dt]

    z0 = nc.dram_tensor("z0", [N, D1 - 1], F32, kind="ExternalInput").ap()
    dtm = nc.dram_tensor("dtm", [N, 1], F32, kind="ExternalInput").ap()
    w1 = nc.dram_tensor("w1", [D1, H], F32, kind="ExternalInput").ap()
    w2 = nc.dram_tensor("w2", [H, D1], F32, kind="ExternalInput").ap()
    b1 = nc.dram_tensor("b1", [H, 1], F32, kind="ExternalInput").ap()
    b2 = nc.dram_tensor("b2", [1, D1], F32, kind="ExternalInput").ap()
    ident = nc.dram_tensor("ident", [D1, D1], F32, kind="ExternalInput").ap()
    yout = nc.dram_tensor("yout", [TS, N, D1], F32, kind="ExternalOutput").ap()

    with tile.TileContext(nc) as tc:
        with (
            tc.tile_pool(name="cpool", bufs=1) as cpool,
            tc.tile_pool(name="spool", bufs=2) as spool,
            tc.tile_pool(name="hpool", bufs=2) as hpool,
            tc.tile_pool(name="opool", bufs=2) as opool,
            tc.tile_pool(name="ypool", bufs=1, space="PSUM") as ypool,
            tc.tile_pool(name="p1pool", bufs=2, space="PSUM") as p1pool,
            tc.tile_pool(name="snpool", bufs=2, space="PSUM") as snpool,
        ):
            # ---- constants / weights ----
            w1s = cpool.tile([D1, H], F32)
            nc.sync.dma_start(w1s[:, :], w1[:, :])
            w2s = cpool.tile([D1, 2, D1], F32)
            nc.sync.dma_start(w2s[:, 0, :], w2[0:128, :])
            nc.sync.dma_start(w2s[:, 1, :], w2[128:256, :])
            # fold the Euler dt into W2 once: y += tanh(...) @ (DT*W2)
            nc.scalar.mul(w2s[:, :, :], w2s[:, :, :], DT)
            if h_dtype != F32:
                w2c = cpool.tile([D1, 2, D1], h_dtype)
                nc.vector.tensor_copy(w2c[:, :, :], w2s[:, :, :])
            else:
                w2c = w2s
            ids = cpool.tile([D1, D1], F32)
            nc.sync.dma_start(ids[:, :], ident[:, :])

            b1s = []
            if not zero_b1:
                for j in range(2):
                    b1t = cpool.tile([D1, 1], F32, name=f"b1_{j}")
                    nc.sync.dma_start(b1t[:, :], b1[128 * j : 128 * (j + 1), :])
                    b1s.append(b1t)
            if not zero_b2:
                b2row = cpool.tile([1, D1], F32)
                nc.sync.dma_start(b2row[:, :], b2[:, :])
                b2dt = cpool.tile([1, D1], F32)
                nc.scalar.mul(b2dt[:, :], b2row[:, :], DT)
                ones = cpool.tile([1, CW], F32)
                nc.vector.memset(ones[:, :], 1.0)

            # ---- per-chain init: y0^T into persistent PSUM, masks ----
            psumY = []
            st = [None] * chains
            masks = []
            for c in range(chains):
                r0, r1 = c * CW, (c + 1) * CW
                y0nat = cpool.tile([CW, D1], F32, name=f"y0nat_{c}")
                nc.sync.dma_start(y0nat[:, 0 : D1 - 1], z0[r0:r1, :])
                nc.sync.dma_start(y0nat[:, D1 - 1 : D1], dtm[r0:r1, :])
                py = ypool.tile([D1, CW], F32, name=f"psumY_{c}")
                nc.tensor.transpose(py[:, :], y0nat[:, :], ids[0:CW, 0:CW])
                psumY.append(py)
                stc = spool.tile([D1, CW], F32, name=f"st_{c}", tag=f"st{c}")
                nc.vector.tensor_copy(stc[:, :], py[:, :])
                st[c] = stc

                dtc = cpool.tile([CW, 1], F32, name=f"dtc_{c}")
                nc.sync.dma_start(dtc[:, :], dtm[r0:r1, :])
                mk = cpool.tile([CW, TS], F32, name=f"mask_{c}")
                for i in range(TS):
                    nc.vector.tensor_scalar(
                        mk[:, i : i + 1],
                        dtc[:, :],
                        float(np.float32(i) / np.float32(10.0)),
                        None,
                        op0=mybir.AluOpType.is_gt,
                    )
                masks.append(mk)

            def snapshot(i: int):
                for c in range(chains):
                    r0, r1 = c * CW, (c + 1) * CW
                    pt = snpool.tile([CW, D1], F32, name=f"pt_{i}_{c}", tag="pt")
                    nc.tensor.transpose(pt[:, :], st[c][:, :], ids[:, :])
                    osb = opool.tile([CW, D1], F32, name=f"osb_{i}_{c}", tag=f"o{c}")
                    nc.vector.tensor_scalar_mul(
                        osb[:, :], pt[:, :], masks[c][:, i : i + 1]
                    )
                    nc.sync.dma_start(yout[i, r0:r1, :], osb[:, :])

            snapshot(0)

            for outer in range(n_outer):
                for k in range(n_steps):
                    p1s = []
                    for c in range(chains):
                        p1 = p1pool.tile(
                            [D1, 2, CW], F32, name=f"p1_{outer}_{k}_{c}", tag=f"p1{c}"
                        )
                        nc.tensor.matmul(
                            p1[:, 0, :], w1s[:, 0:128], st[c][:, :],
                            start=True, stop=True,
                        )
                        nc.tensor.matmul(
                            p1[:, 1, :], w1s[:, 128:256], st[c][:, :],
                            start=True, stop=True,
                        )
                        p1s.append(p1)
                    hs = []
                    for c in range(chains):
                        hshape = [D1, 2, CW]
                        ht = hpool.tile(
                            hshape, h_dtype, name=f"h_{outer}_{k}_{c}", tag=f"h{c}"
                        )
                        if zero_b1:
                            nc.scalar.activation(ht[:, :, :], p1s[c][:, :, :], AF.Tanh)
                        else:
                            for j in range(2):
                                nc.scalar.activation(
                                    ht[:, j, :], p1s[c][:, j, :], AF.Tanh,
                                    bias=b1s[j][:, :],
                                )
                        hs.append(ht)
                        nc.tensor.matmul(
                            psumY[c][:, :], w2c[:, 0, :], ht[:, 0, :],
                            start=False, stop=False, skip_group_check=True,
                        )
                        nc.tensor.matmul(
                            psumY[c][:, :], w2c[:, 1, :], ht[:, 1, :],
                            start=False, stop=zero_b2, skip_group_check=True,
                        )
                        if not zero_b2:
                            nc.tensor.matmul(
                                psumY[c][:, :], b2dt[:, :], ones[:, :],
                                start=False, stop=True, skip_group_check=True,
                            )
                    for c in range(chains):
                        stc = spool.tile(
                            [D1, CW], F32, name=f"st_{outer}_{k}_{c}", tag=f"st{c}"
                        )
                        nc.vector.tensor_copy(stc[:, :], psumY[c][:, :])
                        st[c] = stc
                snapshot(outer + 1)

    nc.compile()
    return nc


def kernel(z0, disappear_time, t, W1, b1, W2, b2):
    z0 = np.ascontiguousarray(np.asarray(z0, dtype=np.float32))
    disappear_time = np.ascontiguousarray(
        np.asarray(disappear_time, dtype=np.float32)
    )
    W1 = np.ascontiguousarray(np.asarray(W1, dtype=np.float32))
    W2 = np.ascontiguousarray(np.asarray(W2, dtype=np.float32))
    b1 = np.asarray(b1, dtype=np.float32).reshape(H, 1)
    b2 = np.asarray(b2, dtype=np.float32).reshape(1, D1)
    ident = np.eye(D1, dtype=np.float32)

    zero_b1 = not np.any(b1)
    zero_b2 = not np.any(b2)
    nc = build_nc(zero_b1, zero_b2)

    in_maps = []
    for b in range(B):
        in_maps.append(
            {
                "z0": np.ascontiguousarray(z0[b]),
                "dtm": np.ascontiguousarray(disappear_time[b]),
                "w1": W1,
                "w2": W2,
                "b1": b1,
                "b2": b2,
                "ident": ident,
            }
        )
    res = run_bass_kernel_spmd(nc, in_maps, core_ids=list(range(B)))
    out = np.stack([res.results[b]["yout"] for b in range(B)], axis=0)
    return out.astype(np.float32)


# revision 5
# speedup vs baseline: 10.2081x; 10.2081x over previous
"""Trainium2 Bass kernel for nn_NeuralODEModel (fixed-step Euler neural ODE).

Math (per batch b, all rows n independent):
  y0 = concat([z0, disappear_time], -1)            # [N, D1]
  repeat 9x: 120 Euler steps y += DT * (tanh(y@W1 + b1) @ W2 + b2)
  out[i] = y_after_{120*i}_steps * (i/10 < disappear_time)   # i = 0..9

Sharding: data-parallel across B=8 -> one batch per NeuronCore (SPMD).

Per-core kernel design:
  - State is kept TRANSPOSED in SBUF/PSUM: ST = y^T [D1=128 part, n free],
    so both matmuls contract over the partition dim with weights stationary:
      mm1: psum1[:,j,:] = W1[:,128j:128j+128].T @ ST     (j = 0,1 -> H=256)
      tanh: h = tanh(psum1 (+b1))          (one ACT op over [128, 2, n])
      mm2: psumY += (DT*W2)[128j:,:].T @ h[:,j,:]        (accumulate onto y^T)
      copy: ST' = psumY                    (DVE PSUM->SBUF, rhs for next step)
    psumY is a persistent PSUM accumulator initialized with y0^T by a PE
    transpose, so y^T lives in PSUM and every step just accumulates into it.
  - C row-chains (columns of ST) are stepped in an interleaved order so the
    serial mm1->tanh->mm2->copy dependency of one chain hides under the
    engine work of the others.
  - Snapshots (every 120 steps + t=0): PE-transpose ST back to natural
    [n, D1], multiply by the precomputed per-row mask (DVE tensor_scalar
    with a per-partition operand), DMA to the output.
"""

import os

import numpy as np

import concourse.bacc as bacc
import concourse.mybir as mybir
from concourse import tile
from concourse.bass_utils import run_bass_kernel_spmd

F32 = mybir.dt.float32
AF = mybir.ActivationFunctionType

B, N, D1, H, TS = 8, 128, 128, 256, 10
DT = 1.0 / 1200.0
STEPS_PER_INT = 120

NUM_CHAINS = int(os.environ.get("NODE_CHAINS", "2"))
MM2_DT = os.environ.get("NODE_MM2_DT", "f32")  # f32 | f16 | bf16
_MM2_DTYPE = {
    "f32": mybir.dt.float32,
    "f16": mybir.dt.float16,
    "bf16": mybir.dt.bfloat16,
}


def build_nc(
    zero_b1: bool,
    zero_b2: bool,
    n_outer: int = TS - 1,
    n_steps: int = STEPS_PER_INT,
    chains: int = NUM_CHAINS,
    mm2_dt: str = MM2_DT,
    work_mult: int = 1,
):
    """Build the per-core SPMD Bass program. Returns a compiled Bacc."""
    nc = bacc.Bacc()
    CW = N // chains  # rows per chain
    h_dtype = _MM2_DTYPE[mm2_dt]

    z0 = nc.dram_tensor("z0", [N, D1 - 1], F32, kind="ExternalInput").ap()
    dtm = nc.dram_tensor("dtm", [N, 1], F32, kind="ExternalInput").ap()
    w1 = nc.dram_tensor("w1", [D1, H], F32, kind="ExternalInput").ap()
    w2 = nc.dram_tensor("w2", [H, D1], F32, kind="ExternalInput").ap()
    b1 = nc.dram_tensor("b1", [H, 1], F32, kind="ExternalInput").ap()
    b2 = nc.dram_tensor("b2", [1, D1], F32, kind="ExternalInput").ap()
    ident = nc.dram_tensor("ident", [D1, D1], F32, kind="ExternalInput").ap()
    yout = nc.dram_tensor("yout", [TS, N, D1], F32, kind="ExternalOutput").ap()

    with tile.TileContext(nc) as tc:
        with (
            tc.tile_pool(name="cpool", bufs=1) as cpool,
            tc.tile_pool(name="spool", bufs=2) as spool,
            tc.tile_pool(name="hpool", bufs=2) as hpool,
            tc.tile_pool(name="opool", bufs=2) as opool,
            tc.tile_pool(name="ypool", bufs=1, space="PSUM") as ypool,
            tc.tile_pool(name="p1pool", bufs=2, space="PSUM") as p1pool,
            tc.tile_pool(name="snpool", bufs=2, space="PSUM") as snpool,
        ):
            # ---- constants / weights ----
            w1s = cpool.tile([D1, H], F32)
            nc.sync.dma_start(w1s[:, :], w1[:, :])
            w2s = cpool.tile([D1, 2, D1], F32)
            nc.sync.dma_start(w2s[:, 0, :], w2[0:128, :])
            nc.sync.dma_start(w2s[:, 1, :], w2[128:256, :])
            # fold the Euler dt into W2 once: y += tanh(...) @ (DT*W2)
            nc.scalar.mul(w2s[:, :, :], w2s[:, :, :], DT)
            if h_dtype != F32:
                w2c = cpool.tile([D1, 2, D1], h_dtype)
                nc.vector.tensor_copy(w2c[:, :, :], w2s[:, :, :])
            else:
                w2c = w2s
            ids = cpool.tile([D1, D1], F32)
            nc.sync.dma_start(ids[:, :], ident[:, :])

            b1s = []
            if not zero_b1:
                for j in range(2):
                    b1t = cpool.tile([D1, 1], F32, name=f"b1_{j}")
                    nc.sync.dma_start(b1t[:, :], b1[128 * j : 128 * (j + 1), :])
                    b1s.append(b1t)
            if not zero_b2:
                b2row = cpool.tile([1, D1], F32)
                nc.sync.dma_start(b2row[:, :], b2[:, :])
                b2dt = cpool.tile([1, D1], F32)
                nc.scalar.mul(b2dt[:, :], b2row[:, :], DT)
                ones = cpool.tile([1, CW], F32)
                nc.vector.memset(ones[:, :], 1.0)

            # ---- per-chain init: y0^T into persistent PSUM, masks ----
            psumY = []
            st = [None] * chains
            masks = []
            for c in range(chains):
                r0, r1 = c * CW, (c + 1) * CW
                y0nat = cpool.tile([CW, D1], F32, name=f"y0nat_{c}")
                nc.sync.dma_start(y0nat[:, 0 : D1 - 1], z0[r0:r1, :])
                nc.sync.dma_start(y0nat[:, D1 - 1 : D1], dtm[r0:r1, :])
                py = ypool.tile([D1, CW], F32, name=f"psumY_{c}")
                nc.tensor.transpose(py[:, :], y0nat[:, :], ids[0:CW, 0:CW])
                psumY.append(py)
                stc = spool.tile([D1, CW], F32, name=f"st_{c}", tag=f"st{c}")
                nc.vector.tensor_copy(stc[:, :], py[:, :])
                st[c] = stc

                dtc = cpool.tile([CW, 1], F32, name=f"dtc_{c}")
                nc.sync.dma_start(dtc[:, :], dtm[r0:r1, :])
                mk = cpool.tile([CW, TS], F32, name=f"mask_{c}")
                for i in range(TS):
                    nc.vector.tensor_scalar(
                        mk[:, i : i + 1],
                        dtc[:, :],
                        float(np.float32(i) / np.float32(10.0)),
                        None,
                        op0=mybir.AluOpType.is_gt,
                    )
                masks.append(mk)

            def snapshot(i: int):
                for c in range(chains):
                    r0, r1 = c * CW, (c + 1) * CW
                    pt = snpool.tile([CW, D1], F32, name=f"pt_{i}_{c}", tag="pt")
                    nc.tensor.transpose(pt[:, :], st[c][:, :], ids[:, :])
                    osb = opool.tile([CW, D1], F32, name=f"osb_{i}_{c}", tag=f"o{c}")
                    nc.vector.tensor_scalar_mul(
                        osb[:, :], pt[:, :], masks[c][:, i : i + 1]
                    )
                    nc.sync.dma_start(yout[i, r0:r1, :], osb[:, :])

            snapshot(0)

            for outer in range(n_outer * work_mult):
                for k in range(n_steps):
                    p1s = []
                    for c in range(chains):
                        p1 = p1pool.tile(
                            [D1, 2, CW], F32, name=f"p1_{outer}_{k}_{c}", tag=f"p1{c}"
                        )
                        nc.tensor.matmul(
                            p1[:, 0, :], w1s[:, 0:128], st[c][:, :],
                            start=True, stop=True,
                        )
                        nc.tensor.matmul(
                            p1[:, 1, :], w1s[:, 128:256], st[c][:, :],
                            start=True, stop=True,
                        )
                        p1s.append(p1)
                    hs = []
                    for c in range(chains):
                        hshape = [D1, 2, CW]
                        ht = hpool.tile(
                            hshape, h_dtype, name=f"h_{outer}_{k}_{c}", tag=f"h{c}"
                        )
                        if zero_b1:
                            nc.scalar.activation(ht[:, :, :], p1s[c][:, :, :], AF.Tanh)
                        else:
                            for j in range(2):
                                nc.scalar.activation(
                                    ht[:, j, :], p1s[c][:, j, :], AF.Tanh,
                                    bias=b1s[j][:, :],
                                )
                        hs.append(ht)
                        nc.tensor.matmul(
                            psumY[c][:, :], w2c[:, 0, :], ht[:, 0, :],
                            start=False, stop=False, skip_group_check=True,
                        )
                        nc.tensor.matmul(
                            psumY[c][:, :], w2c[:, 1, :], ht[:, 1, :],
                            start=False, stop=zero_b2, skip_group_check=True,
                        )
                        if not zero_b2:
                            nc.tensor.matmul(
                                psumY[c][:, :], b2dt[:, :], ones[:, :],
                                start=False, stop=True, skip_group_check=True,
                            )
                    for c in range(chains):
                        stc = spool.tile(
                            [D1, CW], F32, name=f"st_{outer}_{k}_{c}", tag=f"st{c}"
                        )
                        nc.vector.tensor_copy(stc[:, :], psumY[c][:, :])
                        st[c] = stc
                if outer < n_outer:
                    snapshot(min(outer + 1, n_outer))

    nc.compile()
    return nc


def kernel(z0, disappear_time, t, W1, b1, W2, b2):
    z0 = np.ascontiguousarray(np.asarray(z0, dtype=np.float32))
    disappear_time = np.ascontiguousarray(
        np.asarray(disappear_time, dtype=np.float32)
    )
    W1 = np.ascontiguousarray(np.asarray(W1, dtype=np.float32))
    W2 = np.ascontiguousarray(np.asarray(W2, dtype=np.float32))
    b1 = np.asarray(b1, dtype=np.float32).reshape(H, 1)
    b2 = np.asarray(b2, dtype=np.float32).reshape(1, D1)
    ident = np.eye(D1, dtype=np.float32)

    zero_b1 = not np.any(b1)
    zero_b2 = not np.any(b2)
    nc = build_nc(zero_b1, zero_b2)

    in_maps = []
    for b in range(B):
        in_maps.append(
            {
                "z0": np.ascontiguousarray(z0[b]),
                "dtm": np.ascontiguousarray(disappear_time[b]),
                "w1": W1,
                "w2": W2,
                "b1": b1,
                "b2": b2,
                "ident": ident,
            }
        )
    res = run_bass_kernel_spmd(nc, in_maps, core_ids=list(range(B)))
    out = np.stack([res.results[b]["yout"] for b in range(B)], axis=0)
    return out.astype(np.float32)


# revision 12
# speedup vs baseline: 16.1469x; 1.5818x over previous
"""Trainium2 Bass kernel for nn_NeuralODEModel (fixed-step Euler neural ODE).

Math (per batch b, all rows n independent):
  y0 = concat([z0, disappear_time], -1)            # [N, D1]
  repeat 9x: 120 Euler steps y += DT * (tanh(y@W1 + b1) @ W2 + b2)
  out[i] = y_after_{120*i}_steps * (i/10 < disappear_time)   # i = 0..9

Sharding: data-parallel across B=8 -> one batch per NeuronCore (SPMD).

Per-core kernel design:
  - State is kept TRANSPOSED in SBUF/PSUM: ST = y^T [D1=128 part, n free],
    so both matmuls contract over the partition dim with weights stationary:
      mm1: psum1[:,j,:] = W1[:,128j:128j+128].T @ ST     (j = 0,1 -> H=256)
      tanh: h = tanh(psum1 (+b1))          (one ACT op over [128, 2, n])
      mm2: psumY += (DT*W2)[128j:,:].T @ h[:,j,:]        (accumulate onto y^T)
      copy: ST' = psumY                    (DVE PSUM->SBUF, rhs for next step)
    psumY is a persistent PSUM accumulator initialized with y0^T by a PE
    transpose, so y^T lives in PSUM and every step just accumulates into it.
  - C row-chains (columns of ST) are stepped in an interleaved order so the
    serial mm1->tanh->mm2->copy dependency of one chain hides under the
    engine work of the others.
  - Snapshots (every 120 steps + t=0): PE-transpose ST back to natural
    [n, D1], multiply by the precomputed per-row mask (DVE tensor_scalar
    with a per-partition operand), DMA to the output.
"""

import os

import numpy as np

import concourse.bacc as bacc
import concourse.mybir as mybir
from concourse import tile
from concourse.bass_utils import run_bass_kernel_spmd

F32 = mybir.dt.float32
AF = mybir.ActivationFunctionType

B, N, D1, H, TS = 8, 128, 128, 256, 10
DT = 1.0 / 1200.0
STEPS_PER_INT = 120

NUM_CHAINS = int(os.environ.get("NODE_CHAINS", "2"))
MM2_DT = os.environ.get("NODE_MM2_DT", "f32")  # f32 | f16 | bf16
MM1_DT = os.environ.get("NODE_MM1_DT", "f32")  # f32 | f16 | bf16
_DTYPE = {
    "f32": mybir.dt.float32,
    "f16": mybir.dt.float16,
    "bf16": mybir.dt.bfloat16,
}


def build_nc(
    zero_b1: bool,
    zero_b2: bool,
    n_outer: int = TS - 1,
    n_steps: int = STEPS_PER_INT,
    chains: int = NUM_CHAINS,
    mm2_dt: str = MM2_DT,
    mm1_dt: str = MM1_DT,
    work_mult: int = 1,
):
    """Build the per-core SPMD Bass program. Returns a compiled Bacc."""
    nc = bacc.Bacc()
    CW = N // chains  # rows per chain
    h_dtype = _DTYPE[mm2_dt]
    st_dtype = _DTYPE[mm1_dt]

    z0 = nc.dram_tensor("z0", [N, D1 - 1], F32, kind="ExternalInput").ap()
    dtm = nc.dram_tensor("dtm", [N, 1], F32, kind="ExternalInput").ap()
    w1 = nc.dram_tensor("w1", [D1, H], F32, kind="ExternalInput").ap()
    w2 = nc.dram_tensor("w2", [H, D1], F32, kind="ExternalInput").ap()
    b1 = nc.dram_tensor("b1", [H, 1], F32, kind="ExternalInput").ap()
    b2 = nc.dram_tensor("b2", [1, D1], F32, kind="ExternalInput").ap()
    ident = nc.dram_tensor("ident", [D1, D1], F32, kind="ExternalInput").ap()
    yout = nc.dram_tensor("yout", [TS, N, D1], F32, kind="ExternalOutput").ap()

    with tile.TileContext(nc) as tc:
        with (
            tc.tile_pool(name="cpool", bufs=1) as cpool,
            tc.tile_pool(name="spool", bufs=2) as spool,
            tc.tile_pool(name="hpool", bufs=2) as hpool,
            tc.tile_pool(name="opool", bufs=2) as opool,
            tc.tile_pool(name="ypool", bufs=1, space="PSUM") as ypool,
            tc.tile_pool(name="p1pool", bufs=2, space="PSUM") as p1pool,
            tc.tile_pool(name="snpool", bufs=2, space="PSUM") as snpool,
        ):
            # ---- constants / weights ----
            w1s = cpool.tile([D1, H], F32)
            nc.sync.dma_start(w1s[:, :], w1[:, :])
            if st_dtype != F32:
                w1c = cpool.tile([D1, H], st_dtype)
                nc.vector.tensor_copy(w1c[:, :], w1s[:, :])
            else:
                w1c = w1s
            w2s = cpool.tile([D1, 2, D1], F32)
            nc.sync.dma_start(w2s[:, 0, :], w2[0:128, :])
            nc.sync.dma_start(w2s[:, 1, :], w2[128:256, :])
            # fold the Euler dt into W2 once: y += tanh(...) @ (DT*W2)
            nc.scalar.mul(w2s[:, :, :], w2s[:, :, :], DT)
            if h_dtype != F32:
                w2c = cpool.tile([D1, 2, D1], h_dtype)
                nc.vector.tensor_copy(w2c[:, :, :], w2s[:, :, :])
            else:
                w2c = w2s
            ids = cpool.tile([D1, D1], F32)
            nc.sync.dma_start(ids[:, :], ident[:, :])

            b1s = []
            if not zero_b1:
                for j in range(2):
                    b1t = cpool.tile([D1, 1], F32, name=f"b1_{j}")
                    nc.sync.dma_start(b1t[:, :], b1[128 * j : 128 * (j + 1), :])
                    b1s.append(b1t)
            if not zero_b2:
                b2row = cpool.tile([1, D1], F32)
                nc.sync.dma_start(b2row[:, :], b2[:, :])
                b2dt = cpool.tile([1, D1], F32)
                nc.scalar.mul(b2dt[:, :], b2row[:, :], DT)
                ones = cpool.tile([1, CW], F32)
                nc.vector.memset(ones[:, :], 1.0)

            # ---- per-chain init: y0^T into persistent PSUM, masks ----
            psumY = []
            st = [None] * chains
            masks = []
            for c in range(chains):
                r0, r1 = c * CW, (c + 1) * CW
                y0nat = cpool.tile([CW, D1], F32, name=f"y0nat_{c}")
                nc.sync.dma_start(y0nat[:, 0 : D1 - 1], z0[r0:r1, :])
                nc.sync.dma_start(y0nat[:, D1 - 1 : D1], dtm[r0:r1, :])
                py = ypool.tile([D1, CW], F32, name=f"psumY_{c}")
                nc.tensor.transpose(py[:, :], y0nat[:, :], ids[0:CW, 0:CW])
                psumY.append(py)
                stc = spool.tile([D1, CW], st_dtype, name=f"st_{c}", tag=f"st{c}")
                nc.vector.tensor_copy(stc[:, :], py[:, :])
                st[c] = stc

                dtc = cpool.tile([CW, 1], F32, name=f"dtc_{c}")
                nc.sync.dma_start(dtc[:, :], dtm[r0:r1, :])
                mk = cpool.tile([CW, TS], F32, name=f"mask_{c}")
                for i in range(TS):
                    nc.vector.tensor_scalar(
                        mk[:, i : i + 1],
                        dtc[:, :],
                        float(np.float32(i) / np.float32(10.0)),
                        None,
                        op0=mybir.AluOpType.is_gt,
                    )
                masks.append(mk)

            def snapshot(i: int):
                for c in range(chains):
                    r0, r1 = c * CW, (c + 1) * CW
                    if st_dtype != F32:
                        # ST is low-precision; snapshot from the fp32 PSUM state
                        sf = spool.tile(
                            [D1, CW], F32, name=f"st32_{i}_{c}", tag=f"st32_{c}"
                        )
                        nc.vector.tensor_copy(sf[:, :], psumY[c][:, :])
                        src = sf
                    else:
                        src = st[c]
                    pt = snpool.tile([CW, D1], F32, name=f"pt_{i}_{c}", tag="pt")
                    nc.tensor.transpose(pt[:, :], src[:, :], ids[:, :])
                    osb = opool.tile([CW, D1], F32, name=f"osb_{i}_{c}", tag=f"o{c}")
                    nc.vector.tensor_scalar_mul(
                        osb[:, :], pt[:, :], masks[c][:, i : i + 1]
                    )
                    nc.sync.dma_start(yout[i, r0:r1, :], osb[:, :])

            snapshot(0)

            for outer in range(n_outer * work_mult):
                for k in range(n_steps):
                    p1s = []
                    for c in range(chains):
                        p1 = p1pool.tile(
                            [D1, 2, CW], F32, name=f"p1_{outer}_{k}_{c}", tag=f"p1{c}"
                        )
                        nc.tensor.matmul(
                            p1[:, 0, :], w1c[:, 0:128], st[c][:, :],
                            start=True, stop=True,
                        )
                        nc.tensor.matmul(
                            p1[:, 1, :], w1c[:, 128:256], st[c][:, :],
                            start=True, stop=True,
                        )
                        p1s.append(p1)
                    hs = []
                    for c in range(chains):
                        hshape = [D1, 2, CW]
                        ht = hpool.tile(
                            hshape, h_dtype, name=f"h_{outer}_{k}_{c}", tag=f"h{c}"
                        )
                        if zero_b1:
                            nc.scalar.activation(ht[:, :, :], p1s[c][:, :, :], AF.Tanh)
                        else:
                            for j in range(2):
                                nc.scalar.activation(
                                    ht[:, j, :], p1s[c][:, j, :], AF.Tanh,
                                    bias=b1s[j][:, :],
                                )
                        hs.append(ht)
                        nc.tensor.matmul(
                            psumY[c][:, :], w2c[:, 0, :], ht[:, 0, :],
                            start=False, stop=False, skip_group_check=True,
                        )
                        nc.tensor.matmul(
                            psumY[c][:, :], w2c[:, 1, :], ht[:, 1, :],
                            start=False, stop=zero_b2, skip_group_check=True,
                        )
                        if not zero_b2:
                            nc.tensor.matmul(
                                psumY[c][:, :], b2dt[:, :], ones[:, :],
                                start=False, stop=True, skip_group_check=True,
                            )
                    for c in range(chains):
                        stc = spool.tile(
                            [D1, CW], st_dtype, name=f"st_{outer}_{k}_{c}", tag=f"st{c}"
                        )
                        nc.vector.tensor_copy(stc[:, :], psumY[c][:, :])
                        st[c] = stc
                if outer < n_outer:
                    snapshot(min(outer + 1, n_outer))

    nc.compile()
    return nc


def kernel(z0, disappear_time, t, W1, b1, W2, b2):
    z0 = np.ascontiguousarray(np.asarray(z0, dtype=np.float32))
    disappear_time = np.ascontiguousarray(
        np.asarray(disappear_time, dtype=np.float32)
    )
    W1 = np.ascontiguousarray(np.asarray(W1, dtype=np.float32))
    W2 = np.ascontiguousarray(np.asarray(W2, dtype=np.float32))
    b1 = np.asarray(b1, dtype=np.float32).reshape(H, 1)
    b2 = np.asarray(b2, dtype=np.float32).reshape(1, D1)
    ident = np.eye(D1, dtype=np.float32)

    zero_b1 = not np.any(b1)
    zero_b2 = not np.any(b2)
    nc = build_nc(zero_b1, zero_b2)

    in_maps = []
    for b in range(B):
        in_maps.append(
            {
                "z0": np.ascontiguousarray(z0[b]),
                "dtm": np.ascontiguousarray(disappear_time[b]),
                "w1": W1,
                "w2": W2,
                "b1": b1,
                "b2": b2,
                "ident": ident,
            }
        )
    res = run_bass_kernel_spmd(nc, in_maps, core_ids=list(range(B)))
    out = np.stack([res.results[b]["yout"] for b in range(B)], axis=0)
    return out.astype(np.float32)
